# revision 33
# baseline (speedup 1.0000x reference)
"""Trainium2 Bass kernel for AttnAdaIN (attention-weighted AdaIN).

Shapes (hardcoded): B=4, C=512, H=W=64 -> N=M=4096 tokens, Ck=512.

Math:
  F = Wf@content_key+bf, G = Wg@style_key+bg, Hv = Wh@style+bh
  S = softmax(F^T G)  over style tokens
  mean = S @ Hv^T ; second = S @ (Hv^T)^2
  out = sqrt(relu(second-mean^2)) * mvn(content) + mean

Sharding: 8 cores = (batch b, query-token half h). Each core handles 2048
query tokens of one batch; the style-side (G, V) work is recomputed by both
cores of a batch pair (7.5% overhead, no collectives).

Device algorithm (per core), everything in "transposed" [*, n] layout so no
on-chip transposes are ever needed and the output lands directly in [C, N]:
  - conv1x1s as PE matmuls in float32r (full PE speed, ~1e-4 matmul error)
  - scores^T[m,n] tiles via lhsT=G, rhs=F
  - exp on ACT (no max subtraction: |scores| <= ~30 << 88, fp32-safe)
  - mean^T/second^T via lhsT=V-tiles, rhs=exp(S^T); rowsum via ones-lhsT
  - style tokens processed in 8 blocks of 512; partial PV results
    accumulated into SBUF f32 accumulators (PSUM has only 8 banks)
  - softmax normalization folded in at the end: r=1/rowsum (+1 Newton step),
    broadcast across partitions with a K=1 ones matmul
  - mvn(content) stats via bn_stats/bn_aggr (unbiased var correction)
"""

import numpy as np
from contextlib import ExitStack

import concourse.bass as bass
from concourse import bacc
import concourse.tile as tile
import concourse.mybir as mybir
from concourse.bass_utils import run_bass_kernel_spmd

F32 = mybir.dt.float32
F32R = mybir.dt.float32r
F16 = mybir.dt.float16
AF = mybir.ActivationFunctionType

B, C, HH, WW = 4, 512, 64, 64
N = HH * WW            # 4096 tokens (queries per batch)
M = N                  # style tokens
KC = C // 128          # 4 contraction chunks
NL = N // 2            # 2048 queries per core
NCH = NL // 512        # 4 n-chunks of 512
CT = C // 128          # 4 output channel tiles
MBS = 1024             # style-token block size
NMB = M // MBS         # 4 blocks
MT = MBS // 128        # 8 m-tiles per block
EPS = 1e-5


def build_nc():
    nc = bacc.Bacc("TRN2", target_bir_lowering=False, num_devices=8)

    ck = nc.dram_tensor("ck", [C, NL], F32, kind="ExternalInput")
    sk = nc.dram_tensor("sk", [C, M], F32, kind="ExternalInput")
    sv = nc.dram_tensor("sv", [C, M], F32, kind="ExternalInput")
    cont = nc.dram_tensor("cont", [C, N], F32, kind="ExternalInput")
    cont_nl = nc.dram_tensor("cont_nl", [C, NL], F32, kind="ExternalInput")
    wfT = nc.dram_tensor("wfT", [C, C], F32, kind="ExternalInput")
    wgT = nc.dram_tensor("wgT", [C, C], F32, kind="ExternalInput")
    whT = nc.dram_tensor("whT", [C, C], F32, kind="ExternalInput")
    bfv = nc.dram_tensor("bfv", [C], F32, kind="ExternalInput")
    bgv = nc.dram_tensor("bgv", [C], F32, kind="ExternalInput")
    bhv = nc.dram_tensor("bhv", [C], F32, kind="ExternalInput")
    out = nc.dram_tensor("out", [C, NL], F32, kind="ExternalOutput")

    with ExitStack() as ctx:
        tc = ctx.enter_context(tile.TileContext(nc))
        pconst = ctx.enter_context(tc.tile_pool(name="const", bufs=1))
        pdata = ctx.enter_context(tc.tile_pool(name="data", bufs=1))
        pstage = ctx.enter_context(tc.tile_pool(name="stage", bufs=3))
        pexp = ctx.enter_context(tc.tile_pool(name="exp", bufs=2))
        pfin = ctx.enter_context(tc.tile_pool(name="fin", bufs=5))
        pmm = ctx.enter_context(tc.tile_pool(name="pmm", bufs=2, space="PSUM"))
        ppv = ctx.enter_context(tc.tile_pool(name="ppv", bufs=6, space="PSUM"))

        def stage_dma(src_ap, shape):
            st = pstage.tile(shape, F32, tag="stage", name="stg")
            nc.sync.dma_start(st[:shape[0], :shape[1]], src_ap)
            return st

        # ---------------- constants ----------------
        ones_f = pconst.tile([128, 1], F32, tag="ones_f")
        nc.vector.memset(ones_f[:], 1.0)
        ones_r = pconst.tile([128, 128], F16, tag="ones_r")
        nc.vector.memset(ones_r[:], 1.0)

        bf_sb = pconst.tile([128, CT], F32, tag="bf_sb")
        nc.sync.dma_start(bf_sb[:], bfv.ap().rearrange("(c p) -> p c", p=128))
        bg_sb = pconst.tile([128, CT], F32, tag="bg_sb")
        nc.sync.dma_start(bg_sb[:], bgv.ap().rearrange("(c p) -> p c", p=128))
        bh_ap = bhv.ap()
        bh_bc_ap = bass.AP(tensor=bh_ap.tensor, offset=bh_ap.offset,
                           ap=[[0, 128]] + list(bh_ap.ap))
        bh_bc = pconst.tile([128, C], F32, tag="bh_bc")
        nc.sync.dma_start(bh_bc[:], bh_bc_ap)
        eps_sb = pconst.tile([128, 1], F32, tag="eps_sb")
        nc.vector.memset(eps_sb[:], EPS)
        # exp shift: keeps exp(s - SHIFT) in f16 range for |s| <= ~29.6;
        # cancels exactly in the softmax normalization (r = 1/rowsum)
        shift_sb = pconst.tile([128, 1], F32, tag="shift_sb")
        nc.vector.memset(shift_sb[:], -18.5)

        # ---------------- weights: DMA + round to f32r ----------------
        # wf is dead after the F conv; wg reuses its tag (and space).
        def load_weight_r(tagbase, dram, dt):
            tiles = []
            for kc in range(KC):
                st = stage_dma(dram[kc * 128:(kc + 1) * 128, :], [128, C])
                tr = pdata.tile([128, C], dt, tag=f"{tagbase}_{kc}",
                                name=f"{tagbase}_{kc}")
                nc.vector.tensor_copy(tr[:], st[:, :C])
                tiles.append(tr)
            return tiles

        wf_r = load_weight_r("wf", wfT, F16)
        wg_r = load_weight_r("wg", wgT, F16)
        wh_r = load_weight_r("wh", whT, F16)

        # ------- content stats (mvn); emission deferred to fill slack -------
        neg_mean = []
        cinv = []
        nmc = []

        def emit_stats():
            for ct in range(CT):
                stats = pfin.tile([128, N // 512, 6], F32, tag="bnstats",
                                  name="bnstats")
                for hlf in range(4):
                    cont_t = stage_dma(
                        cont[ct * 128:(ct + 1) * 128,
                             hlf * 1024:(hlf + 1) * 1024],
                        [128, 1024])
                    view = cont_t[:, :1024].rearrange("p (s d) -> p s d", d=512)
                    for s in range(2):
                        nc.vector.bn_stats(stats[:, hlf * 2 + s, :], view[:, s, :])
                mv = pconst.tile([128, 2], F32, tag=f"mv_{ct}", name=f"mv_{ct}")
                nc.vector.bn_aggr(mv[:], stats[:])
                nm = pconst.tile([128, 1], F32, tag=f"nm_{ct}", name=f"nm_{ct}")
                nc.vector.tensor_scalar_mul(nm[:], mv[:, 0:1], -1.0)
                sd = pconst.tile([128, 1], F32, tag=f"sd_{ct}", name=f"sd_{ct}")
                nc.scalar.activation(sd[:], mv[:, 1:2], AF.Sqrt,
                                     scale=float(N) / float(N - 1), bias=eps_sb[:])
                iv = pconst.tile([128, 1], F32, tag=f"iv_{ct}", name=f"iv_{ct}")
                nc.vector.reciprocal(iv[:], sd[:])
                neg_mean.append(nm)
                cinv.append(iv)
                t = pconst.tile([128, 1], F32, tag=f"nmc_{ct}", name=f"nmc_{ct}")
                nc.vector.tensor_mul(t[:], nm[:], iv[:])
                nmc.append(t)

        # ---------------- style-input loaders (hoistable) ----------------
        def load_skr(mb):
            msl = slice(mb * MBS, (mb + 1) * MBS)
            tiles = []
            for kc in range(KC):
                st = stage_dma(sk[kc * 128:(kc + 1) * 128, msl], [128, MBS])
                sr = pexp.tile([128, MBS], F16, tag=f"exps_{kc}",
                               name=f"skr_{kc}")
                nc.vector.tensor_copy(sr[:, :MBS], st[:, :MBS])
                tiles.append(sr)
            return tiles

        # ---------------- F = conv(ck) in [C, NL] f32r ----------------
        f_sb = [pdata.tile([128, NL], F16, tag=f"f_{ct}", name=f"f_{ct}")
                for ct in range(CT)]
        for q in range(NCH):  # process ck in [C, 512] chunks
            qsl = slice(q * 512, (q + 1) * 512)
            ck_r = []
            for kc in range(KC):
                st = stage_dma(ck[kc * 128:(kc + 1) * 128, qsl], [128, 512])
                cr = pexp.tile([128, 512], F16, tag=f"exps_{kc}",
                               name=f"ckr_{kc}")
                nc.vector.tensor_copy(cr[:], st[:, :512])
                ck_r.append(cr)
            for ct in range(CT):
                ps = pmm.tile([128, 512], F32, tag="mm", name="ps_f")
                for kc in range(KC):
                    nc.tensor.matmul(
                        ps[:], wf_r[kc][:, ct * 128:(ct + 1) * 128],
                        ck_r[kc][:],
                        start=(kc == 0), stop=(kc == KC - 1))
                nc.scalar.activation(
                    f_sb[ct][:, qsl],
                    ps[:], AF.Identity, bias=bf_sb[:, ct:ct + 1])


        # ---------------- accumulators ----------------
        # layout: [(nch, ct) blocks of 512] so a 2048-wide psum evac is one op
        accm_all = pdata.tile([128, NCH * CT * 512], F32, tag="accm", name="accm")
        accs_all = pdata.tile([128, NCH * CT * 512], F32, tag="accs", name="accs")
        accr = pdata.tile([128, NL], F32, tag="accr", name="accr")

        def acc_sl(nch):
            return slice(nch * CT * 512, (nch + 1) * CT * 512)

        rb_l = {}

        def emit_rpipe(nch):
            # rowsum arrives pre-broadcast (ones [128,128] lhsT); one wide recip
            nsl = slice(nch * 512, (nch + 1) * 512)
            rb = pconst.tile([128, 512], F32, tag=f"rb_{nch}", name=f"rb_{nch}")
            nc.vector.reciprocal(rb[:], accr[:, nsl])
            rb_l[nch] = rb

        def emit_final(nch):
            nsl = slice(nch * 512, (nch + 1) * 512)
            rb = rb_l[nch]
            # rb repeated across the two 512-blocks of a ct-pair via a
            # step-0 middle axis AP
            rb_ap = rb[:]
            rb_rep = bass.AP(tensor=rb_ap.tensor, offset=rb_ap.offset,
                             ap=[list(rb_ap.ap[0]), [0, 2], list(rb_ap.ap[1])])
            for cp in range(CT // 2):
                psl = slice((nch * CT + 2 * cp) * 512,
                            (nch * CT + 2 * cp + 2) * 512)
                m_w = pfin.tile([128, 1024], F32, tag="finw", name="m_w")
                nc.gpsimd.tensor_mul(
                    m_w[:].rearrange("p (a b) -> p a b", a=2),
                    accm_all[:, psl].rearrange("p (a b) -> p a b", a=2), rb_rep)
                s_w = pfin.tile([128, 1024], F32, tag="finw", name="s_w")
                nc.vector.tensor_mul(
                    s_w[:].rearrange("p (a b) -> p a b", a=2),
                    accs_all[:, psl].rearrange("p (a b) -> p a b", a=2), rb_rep)
                ms_w = pfin.tile([128, 1024], F32, tag="finw", name="ms_w")
                nc.scalar.activation(ms_w[:], m_w[:], AF.Square)
                v_w = pfin.tile([128, 1024], F32, tag="finw", name="v_w")
                nc.vector.tensor_sub(v_w[:], s_w[:], ms_w[:])
                v2_w = pfin.tile([128, 1024], F32, tag="finw", name="v2_w")
                nc.scalar.activation(v2_w[:], v_w[:], AF.Relu)
                std_w = pfin.tile([128, 1024], F32, tag="finw", name="std_w")
                nc.scalar.activation(std_w[:], v2_w[:], AF.Sqrt)
                for h in range(2):
                    ct = 2 * cp + h
                    c_t = stage_dma(cont_nl[ct * 128:(ct + 1) * 128, nsl],
                                    [128, 512])
                    nrm_t = pfin.tile([128, 512], F32, tag="fin5", name="nrm_t")
                    nc.scalar.activation(nrm_t[:], c_t[:, :512], AF.Identity,
                                         bias=nmc[ct][:], scale=cinv[ct][:])
                    o1_t = pfin.tile([128, 512], F32, tag="fin5", name="o1_t")
                    nc.vector.tensor_mul(
                        o1_t[:], std_w[:, h * 512:(h + 1) * 512], nrm_t[:])
                    o_t = pfin.tile([128, 512], F32, tag="fin5", name="o_t")
                    nc.vector.tensor_add(
                        o_t[:], o1_t[:], m_w[:, h * 512:(h + 1) * 512])
                    nc.sync.dma_start(out[ct * 128:(ct + 1) * 128, nsl], o_t[:])

        # ---------------- main loop over style-token blocks ----------------
        for mb in range(NMB):
            msl = slice(mb * MBS, (mb + 1) * MBS)

            # G block [C, MBS] f16
            sk_r = load_skr(mb)
            g_sb = [pdata.tile([128, MBS], F16, tag=f"g_{ct}", name=f"g_{ct}")
                    for ct in range(CT)]
            for ct in range(CT):
                for gh in range(MBS // 512):
                    gsl = slice(gh * 512, (gh + 1) * 512)
                    ps = pmm.tile([128, 512], F32, tag="mm", name="ps_g")
                    for kc in range(KC):
                        nc.tensor.matmul(
                            ps[:], wg_r[kc][:, ct * 128:(ct + 1) * 128],
                            sk_r[kc][:, gsl],
                            start=(kc == 0), stop=(kc == KC - 1))
                    nc.scalar.activation(g_sb[ct][:, gsl], ps[:], AF.Identity,
                                         bias=bg_sb[:, ct:ct + 1])

            # V block: VT [MBS, C] (m on partitions) f32r, and VT^2,
            # processed in 512-token halves so svr fits the shared tags
            vt = [pdata.tile([128, C], F16, tag=f"vt_{mt}", name=f"vt_{mt}")
                  for mt in range(MT)]
            vtsq = [pdata.tile([128, C], F16, tag=f"vtsq_{mt}",
                               name=f"vtsq_{mt}") for mt in range(MT)]
            for vh in range(MBS // 512):
                sv_r = []
                for kc in range(KC):
                    st = stage_dma(
                        sv[kc * 128:(kc + 1) * 128,
                           mb * MBS + vh * 512:mb * MBS + (vh + 1) * 512],
                        [128, 512])
                    sr = pexp.tile([128, 512], F16, tag=f"exps_{kc + 4}",
                                   name=f"svr_{kc}")
                    nc.vector.tensor_copy(sr[:, :512], st[:, :512])
                    sv_r.append(sr)
                for mt2 in range(4):
                    mt = vh * 4 + mt2
                    ps = pmm.tile([128, 512], F32, tag="mm", name="ps_v")
                    for kc in range(KC):
                        nc.tensor.matmul(
                            ps[:], sv_r[kc][:, mt2 * 128:(mt2 + 1) * 128],
                            wh_r[kc][:],
                            start=(kc == 0), stop=(kc == KC - 1))
                    nc.vector.tensor_add(vt[mt][:], ps[:], bh_bc[:])
                    nc.vector.tensor_mul(vtsq[mt][:], vt[mt][:], vt[mt][:])

            # n-chunks: scores -> exp -> PV passes
            for nch in range(NCH):
                nsl = slice(nch * 512, (nch + 1) * 512)

                exps = [pexp.tile([128, 512], F16, tag=f"exps_{mt}",
                                  name=f"exps_{mt}") for mt in range(MT)]
                # MT == 8 -> tags exps_0..7
                for mt in range(MT):
                    ps = pmm.tile([128, 512], F32, tag="mm", name="ps_s")
                    for kc in range(KC):
                        nc.tensor.matmul(
                            ps[:], g_sb[kc][:, mt * 128:(mt + 1) * 128],
                            f_sb[kc][:, nsl],
                            start=(kc == 0), stop=(kc == KC - 1))
                    nc.scalar.activation(exps[mt][:], ps[:], AF.Exp, bias=shift_sb[:])

                # pass 1: mean + rowsum
                pvm = [ppv.tile([128, 512], F32, tag="pv", name="pvm")
                       for _ in range(CT)]
                pr = ppv.tile([128, 512], F32, tag="pv", name="pr")
                for mt in range(MT):
                    for ct in range(CT):
                        nc.tensor.matmul(
                            pvm[ct][:],
                            vt[mt][:, ct * 128:(ct + 1) * 128],
                            exps[mt][:],
                            start=(mt == 0), stop=(mt == MT - 1))
                    nc.tensor.matmul(pr[:], ones_r[:], exps[mt][:],
                                     start=(mt == 0), stop=(mt == MT - 1))
                for ct in range(CT):
                    asl = slice((nch * CT + ct) * 512, (nch * CT + ct + 1) * 512)
                    if mb == 0:
                        nc.scalar.activation(accm_all[:, asl], pvm[ct][:], AF.Copy)
                    else:
                        nc.vector.tensor_add(accm_all[:, asl], pvm[ct][:],
                                             accm_all[:, asl])
                if mb == 0:
                    nc.scalar.activation(accr[:, nsl], pr[:], AF.Copy)
                else:
                    nc.vector.tensor_add(accr[:, nsl], pr[:], accr[:, nsl])

                # pass 2: second moment
                pvs = [ppv.tile([128, 512], F32, tag="pv", name="pvs")
                       for _ in range(CT)]
                for mt in range(MT):
                    for ct in range(CT):
                        nc.tensor.matmul(
                            pvs[ct][:],
                            vtsq[mt][:, ct * 128:(ct + 1) * 128],
                            exps[mt][:],
                            start=(mt == 0), stop=(mt == MT - 1))
                for ct in range(CT):
                    asl = slice((nch * CT + ct) * 512, (nch * CT + ct + 1) * 512)
                    if mb == 0:
                        nc.scalar.activation(accs_all[:, asl], pvs[ct][:], AF.Copy)
                    else:
                        nc.vector.tensor_add(accs_all[:, asl], pvs[ct][:],
                                             accs_all[:, asl])
                if mb == NMB - 1:
                    emit_rpipe(nch)
            if mb == 1:
                emit_stats()

        for nch in range(NCH):
            emit_final(nch)

    nc.finalize()
    return nc


_NC = None
_last_in_maps = None


def _get_nc():
    global _NC
    if _NC is None:
        _NC = build_nc()
    return _NC


def kernel(content, style, content_key, style_key, Wf, bf, Wg, bg, Wh, bh):
    content = np.ascontiguousarray(np.asarray(content, np.float32))
    style = np.ascontiguousarray(np.asarray(style, np.float32))
    content_key = np.ascontiguousarray(np.asarray(content_key, np.float32))
    style_key = np.ascontiguousarray(np.asarray(style_key, np.float32))
    wfT = np.ascontiguousarray(np.asarray(Wf, np.float32).T)
    wgT = np.ascontiguousarray(np.asarray(Wg, np.float32).T)
    whT = np.ascontiguousarray(np.asarray(Wh, np.float32).T)
    bf = np.ascontiguousarray(np.asarray(bf, np.float32))
    bg = np.ascontiguousarray(np.asarray(bg, np.float32))
    bh = np.ascontiguousarray(np.asarray(bh, np.float32))

    nc = _get_nc()
    global _last_in_maps
    in_maps = []
    for core in range(8):
        b = core // 2
        h = core % 2
        nsl = slice(h * NL, (h + 1) * NL)
        in_maps.append({
            "ck": np.ascontiguousarray(content_key[b].reshape(C, N)[:, nsl]),
            "sk": np.ascontiguousarray(style_key[b].reshape(C, M)),
            "sv": np.ascontiguousarray(style[b].reshape(C, M)),
            "cont": np.ascontiguousarray(content[b].reshape(C, N)),
            "cont_nl": np.ascontiguousarray(content[b].reshape(C, N)[:, nsl]),
            "wfT": wfT, "wgT": wgT, "whT": whT,
            "bfv": bf, "bgv": bg, "bhv": bh,
        })
    _last_in_maps = in_maps
    res = run_bass_kernel_spmd(nc, in_maps, list(range(8)))

    out_full = np.empty((B, C, HH, WW), np.float32)
    for core in range(8):
        b = core // 2
        h = core % 2
        out_full[b].reshape(C, N)[:, h * NL:(h + 1) * NL] = res.results[core]["out"]
    return out_full


# revision 34
# speedup vs baseline: 1.0171x; 1.0171x over previous
"""Trainium2 Bass kernel for AttnAdaIN (attention-weighted AdaIN).

Shapes (hardcoded): B=4, C=512, H=W=64 -> N=M=4096 tokens, Ck=512.

Math:
  F = Wf@content_key+bf, G = Wg@style_key+bg, Hv = Wh@style+bh
  S = softmax(F^T G)  over style tokens
  mean = S @ Hv^T ; second = S @ (Hv^T)^2
  out = sqrt(relu(second-mean^2)) * mvn(content) + mean

Sharding: 8 cores = (batch b, query-token half h). Each core handles 2048
query tokens of one batch; the style-side (G, V) work is recomputed by both
cores of a batch pair (7.5% overhead, no collectives).

Device algorithm (per core), everything in "transposed" [*, n] layout so no
on-chip transposes are ever needed and the output lands directly in [C, N]:
  - conv1x1s as PE matmuls in float32r (full PE speed, ~1e-4 matmul error)
  - scores^T[m,n] tiles via lhsT=G, rhs=F
  - exp on ACT (no max subtraction: |scores| <= ~30 << 88, fp32-safe)
  - mean^T/second^T via lhsT=V-tiles, rhs=exp(S^T); rowsum via ones-lhsT
  - style tokens processed in 8 blocks of 512; partial PV results
    accumulated into SBUF f32 accumulators (PSUM has only 8 banks)
  - softmax normalization folded in at the end: r=1/rowsum (+1 Newton step),
    broadcast across partitions with a K=1 ones matmul
  - mvn(content) stats via bn_stats/bn_aggr (unbiased var correction)
"""

import numpy as np
from contextlib import ExitStack

import concourse.bass as bass
from concourse import bacc
import concourse.tile as tile
import concourse.mybir as mybir
from concourse.bass_utils import run_bass_kernel_spmd

F32 = mybir.dt.float32
F32R = mybir.dt.float32r
F16 = mybir.dt.float16
AF = mybir.ActivationFunctionType

B, C, HH, WW = 4, 512, 64, 64
N = HH * WW            # 4096 tokens (queries per batch)
M = N                  # style tokens
KC = C // 128          # 4 contraction chunks
NL = N // 2            # 2048 queries per core
NCH = NL // 512        # 4 n-chunks of 512
CT = C // 128          # 4 output channel tiles
MBS = 1024             # style-token block size
NMB = M // MBS         # 4 blocks
MT = MBS // 128        # 8 m-tiles per block
EPS = 1e-5


def build_nc():
    nc = bacc.Bacc("TRN2", target_bir_lowering=False, num_devices=8)

    ck = nc.dram_tensor("ck", [C, NL], F32, kind="ExternalInput")
    sk = nc.dram_tensor("sk", [C, M], F32, kind="ExternalInput")
    sv = nc.dram_tensor("sv", [C, M], F32, kind="ExternalInput")
    cont = nc.dram_tensor("cont", [C, N], F32, kind="ExternalInput")
    cont_nl = nc.dram_tensor("cont_nl", [C, NL], F32, kind="ExternalInput")
    wfT = nc.dram_tensor("wfT", [C, C], F32, kind="ExternalInput")
    wgT = nc.dram_tensor("wgT", [C, C], F32, kind="ExternalInput")
    whT = nc.dram_tensor("whT", [C, C], F32, kind="ExternalInput")
    bfv = nc.dram_tensor("bfv", [C], F32, kind="ExternalInput")
    bgv = nc.dram_tensor("bgv", [C], F32, kind="ExternalInput")
    bhv = nc.dram_tensor("bhv", [C], F32, kind="ExternalInput")
    out = nc.dram_tensor("out", [C, NL], F32, kind="ExternalOutput")

    with ExitStack() as ctx:
        tc = ctx.enter_context(tile.TileContext(nc))
        pconst = ctx.enter_context(tc.tile_pool(name="const", bufs=1))
        pdata = ctx.enter_context(tc.tile_pool(name="data", bufs=1))
        pstage = ctx.enter_context(tc.tile_pool(name="stage", bufs=3))
        pexp = ctx.enter_context(tc.tile_pool(name="exp", bufs=2))
        pfin = ctx.enter_context(tc.tile_pool(name="fin", bufs=5))
        pmm = ctx.enter_context(tc.tile_pool(name="pmm", bufs=2, space="PSUM"))
        ppv = ctx.enter_context(tc.tile_pool(name="ppv", bufs=6, space="PSUM"))

        def stage_dma(src_ap, shape):
            st = pstage.tile(shape, F32, tag="stage", name="stg")
            nc.sync.dma_start(st[:shape[0], :shape[1]], src_ap)
            return st

        # ---------------- constants ----------------
        ones_f = pconst.tile([128, 1], F32, tag="ones_f")
        nc.vector.memset(ones_f[:], 1.0)
        ones_r = pconst.tile([128, 128], F16, tag="ones_r")
        nc.vector.memset(ones_r[:], 1.0)

        bf_sb = pconst.tile([128, CT], F32, tag="bf_sb")
        nc.sync.dma_start(bf_sb[:], bfv.ap().rearrange("(c p) -> p c", p=128))
        bg_sb = pconst.tile([128, CT], F32, tag="bg_sb")
        nc.sync.dma_start(bg_sb[:], bgv.ap().rearrange("(c p) -> p c", p=128))
        bh_ap = bhv.ap()
        bh_bc_ap = bass.AP(tensor=bh_ap.tensor, offset=bh_ap.offset,
                           ap=[[0, 128]] + list(bh_ap.ap))
        bh_bc = pconst.tile([128, C], F32, tag="bh_bc")
        nc.sync.dma_start(bh_bc[:], bh_bc_ap)
        eps_sb = pconst.tile([128, 1], F32, tag="eps_sb")
        nc.vector.memset(eps_sb[:], EPS)
        # exp shift: keeps exp(s - SHIFT) in f16 range for |s| <= ~29.6;
        # cancels exactly in the softmax normalization (r = 1/rowsum)
        shift_sb = pconst.tile([128, 1], F32, tag="shift_sb")
        nc.vector.memset(shift_sb[:], -18.5)

        # ---------------- weights: DMA + round to f32r ----------------
        # wf is dead after the F conv; wg reuses its tag (and space).
        def load_weight_r(tagbase, dram, dt):
            tiles = []
            for kc in range(KC):
                st = stage_dma(dram[kc * 128:(kc + 1) * 128, :], [128, C])
                tr = pdata.tile([128, C], dt, tag=f"{tagbase}_{kc}",
                                name=f"{tagbase}_{kc}")
                nc.vector.tensor_copy(tr[:], st[:, :C])
                tiles.append(tr)
            return tiles

        wf_r = load_weight_r("wf", wfT, F16)
        wg_r = load_weight_r("wg", wgT, F16)
        wh_r = load_weight_r("wh", whT, F16)

        # ------- content stats (mvn); emission deferred to fill slack -------
        neg_mean = []
        cinv = []
        nmc = []

        def emit_stats(cts):
            for ct in cts:
                stats = pfin.tile([128, N // 512, 6], F32, tag="bnstats",
                                  name="bnstats")
                for hlf in range(4):
                    cont_t = stage_dma(
                        cont[ct * 128:(ct + 1) * 128,
                             hlf * 1024:(hlf + 1) * 1024],
                        [128, 1024])
                    view = cont_t[:, :1024].rearrange("p (s d) -> p s d", d=512)
                    for s in range(2):
                        nc.vector.bn_stats(stats[:, hlf * 2 + s, :], view[:, s, :])
                mv = pconst.tile([128, 2], F32, tag=f"mv_{ct}", name=f"mv_{ct}")
                nc.vector.bn_aggr(mv[:], stats[:])
                nm = pconst.tile([128, 1], F32, tag=f"nm_{ct}", name=f"nm_{ct}")
                nc.vector.tensor_scalar_mul(nm[:], mv[:, 0:1], -1.0)
                sd = pconst.tile([128, 1], F32, tag=f"sd_{ct}", name=f"sd_{ct}")
                nc.scalar.activation(sd[:], mv[:, 1:2], AF.Sqrt,
                                     scale=float(N) / float(N - 1), bias=eps_sb[:])
                iv = pconst.tile([128, 1], F32, tag=f"iv_{ct}", name=f"iv_{ct}")
                nc.vector.reciprocal(iv[:], sd[:])
                neg_mean.append(nm)
                cinv.append(iv)
                t = pconst.tile([128, 1], F32, tag=f"nmc_{ct}", name=f"nmc_{ct}")
                nc.vector.tensor_mul(t[:], nm[:], iv[:])
                nmc.append(t)

        # ---------------- style-input loaders (hoistable) ----------------
        def load_skr(mb):
            msl = slice(mb * MBS, (mb + 1) * MBS)
            tiles = []
            for kc in range(KC):
                st = stage_dma(sk[kc * 128:(kc + 1) * 128, msl], [128, MBS])
                sr = pexp.tile([128, MBS], F16, tag=f"exps_{kc}",
                               name=f"skr_{kc}")
                nc.vector.tensor_copy(sr[:, :MBS], st[:, :MBS])
                tiles.append(sr)
            return tiles

        # ---------------- F = conv(ck) in [C, NL] f32r ----------------
        f_sb = [pdata.tile([128, NL], F16, tag=f"f_{ct}", name=f"f_{ct}")
                for ct in range(CT)]
        for q in range(NCH):  # process ck in [C, 512] chunks
            qsl = slice(q * 512, (q + 1) * 512)
            ck_r = []
            for kc in range(KC):
                st = stage_dma(ck[kc * 128:(kc + 1) * 128, qsl], [128, 512])
                cr = pexp.tile([128, 512], F16, tag=f"exps_{kc}",
                               name=f"ckr_{kc}")
                nc.vector.tensor_copy(cr[:], st[:, :512])
                ck_r.append(cr)
            for ct in range(CT):
                ps = pmm.tile([128, 512], F32, tag="mm", name="ps_f")
                for kc in range(KC):
                    nc.tensor.matmul(
                        ps[:], wf_r[kc][:, ct * 128:(ct + 1) * 128],
                        ck_r[kc][:],
                        start=(kc == 0), stop=(kc == KC - 1))
                nc.scalar.activation(
                    f_sb[ct][:, qsl],
                    ps[:], AF.Identity, bias=bf_sb[:, ct:ct + 1])


        # ---------------- accumulators ----------------
        # layout: [(nch, ct) blocks of 512] so a 2048-wide psum evac is one op
        accm_all = pdata.tile([128, NCH * CT * 512], F32, tag="accm", name="accm")
        accs_all = pdata.tile([128, NCH * CT * 512], F32, tag="accs", name="accs")
        accr = pdata.tile([128, NL], F32, tag="accr", name="accr")

        def acc_sl(nch):
            return slice(nch * CT * 512, (nch + 1) * CT * 512)

        rb_l = {}

        def emit_rpipe(nch):
            # rowsum arrives pre-broadcast (ones [128,128] lhsT); one wide recip
            nsl = slice(nch * 512, (nch + 1) * 512)
            rb = pconst.tile([128, 512], F32, tag=f"rb_{nch}", name=f"rb_{nch}")
            nc.vector.reciprocal(rb[:], accr[:, nsl])
            rb_l[nch] = rb

        def emit_final(nch):
            nsl = slice(nch * 512, (nch + 1) * 512)
            rb = rb_l[nch]
            # rb repeated across the two 512-blocks of a ct-pair via a
            # step-0 middle axis AP
            rb_ap = rb[:]
            rb_rep = bass.AP(tensor=rb_ap.tensor, offset=rb_ap.offset,
                             ap=[list(rb_ap.ap[0]), [0, 2], list(rb_ap.ap[1])])
            for cp in range(CT // 2):
                psl = slice((nch * CT + 2 * cp) * 512,
                            (nch * CT + 2 * cp + 2) * 512)
                m_w = pfin.tile([128, 1024], F32, tag="finw", name="m_w")
                nc.gpsimd.tensor_mul(
                    m_w[:].rearrange("p (a b) -> p a b", a=2),
                    accm_all[:, psl].rearrange("p (a b) -> p a b", a=2), rb_rep)
                s_w = pfin.tile([128, 1024], F32, tag="finw", name="s_w")
                nc.vector.tensor_mul(
                    s_w[:].rearrange("p (a b) -> p a b", a=2),
                    accs_all[:, psl].rearrange("p (a b) -> p a b", a=2), rb_rep)
                ms_w = pfin.tile([128, 1024], F32, tag="finw", name="ms_w")
                nc.scalar.activation(ms_w[:], m_w[:], AF.Square)
                v_w = pfin.tile([128, 1024], F32, tag="finw", name="v_w")
                nc.vector.tensor_sub(v_w[:], s_w[:], ms_w[:])
                v2_w = pfin.tile([128, 1024], F32, tag="finw", name="v2_w")
                nc.scalar.activation(v2_w[:], v_w[:], AF.Relu)
                std_w = pfin.tile([128, 1024], F32, tag="finw", name="std_w")
                nc.scalar.activation(std_w[:], v2_w[:], AF.Sqrt)
                for h in range(2):
                    ct = 2 * cp + h
                    c_t = stage_dma(cont_nl[ct * 128:(ct + 1) * 128, nsl],
                                    [128, 512])
                    nrm_t = pfin.tile([128, 512], F32, tag="fin5", name="nrm_t")
                    nc.scalar.activation(nrm_t[:], c_t[:, :512], AF.Identity,
                                         bias=nmc[ct][:], scale=cinv[ct][:])
                    o1_t = pfin.tile([128, 512], F32, tag="fin5", name="o1_t")
                    nc.vector.tensor_mul(
                        o1_t[:], std_w[:, h * 512:(h + 1) * 512], nrm_t[:])
                    o_t = pfin.tile([128, 512], F32, tag="fin5", name="o_t")
                    nc.vector.tensor_add(
                        o_t[:], o1_t[:], m_w[:, h * 512:(h + 1) * 512])
                    nc.sync.dma_start(out[ct * 128:(ct + 1) * 128, nsl], o_t[:])

        # ---------------- main loop over style-token blocks ----------------
        for mb in range(NMB):
            msl = slice(mb * MBS, (mb + 1) * MBS)

            # G block [C, MBS] f16
            sk_r = load_skr(mb)
            g_sb = [pdata.tile([128, MBS], F16, tag=f"g_{ct}", name=f"g_{ct}")
                    for ct in range(CT)]
            for ct in range(CT):
                for gh in range(MBS // 512):
                    gsl = slice(gh * 512, (gh + 1) * 512)
                    ps = pmm.tile([128, 512], F32, tag="mm", name="ps_g")
                    for kc in range(KC):
                        nc.tensor.matmul(
                            ps[:], wg_r[kc][:, ct * 128:(ct + 1) * 128],
                            sk_r[kc][:, gsl],
                            start=(kc == 0), stop=(kc == KC - 1))
                    nc.scalar.activation(g_sb[ct][:, gsl], ps[:], AF.Identity,
                                         bias=bg_sb[:, ct:ct + 1])

            # V block: VT [MBS, C] (m on partitions) f32r, and VT^2,
            # processed in 512-token halves so svr fits the shared tags
            vt = [pdata.tile([128, C], F16, tag=f"vt_{mt}", name=f"vt_{mt}")
                  for mt in range(MT)]
            vtsq = [pdata.tile([128, C], F16, tag=f"vtsq_{mt}",
                               name=f"vtsq_{mt}") for mt in range(MT)]
            for vh in range(MBS // 512):
                sv_r = []
                for kc in range(KC):
                    st = stage_dma(
                        sv[kc * 128:(kc + 1) * 128,
                           mb * MBS + vh * 512:mb * MBS + (vh + 1) * 512],
                        [128, 512])
                    sr = pexp.tile([128, 512], F16, tag=f"exps_{kc + 4}",
                                   name=f"svr_{kc}")
                    nc.vector.tensor_copy(sr[:, :512], st[:, :512])
                    sv_r.append(sr)
                for mt2 in range(4):
                    mt = vh * 4 + mt2
                    ps = pmm.tile([128, 512], F32, tag="mm", name="ps_v")
                    for kc in range(KC):
                        nc.tensor.matmul(
                            ps[:], sv_r[kc][:, mt2 * 128:(mt2 + 1) * 128],
                            wh_r[kc][:],
                            start=(kc == 0), stop=(kc == KC - 1))
                    nc.vector.tensor_add(vt[mt][:], ps[:], bh_bc[:])
                    nc.vector.tensor_mul(vtsq[mt][:], vt[mt][:], vt[mt][:])

            # n-chunks: scores -> exp -> PV passes
            for nch in range(NCH):
                nsl = slice(nch * 512, (nch + 1) * 512)

                exps = [pexp.tile([128, 512], F16, tag=f"exps_{mt}",
                                  name=f"exps_{mt}") for mt in range(MT)]
                # MT == 8 -> tags exps_0..7
                for mt in range(MT):
                    ps = pmm.tile([128, 512], F32, tag="mm", name="ps_s")
                    for kc in range(KC):
                        nc.tensor.matmul(
                            ps[:], g_sb[kc][:, mt * 128:(mt + 1) * 128],
                            f_sb[kc][:, nsl],
                            start=(kc == 0), stop=(kc == KC - 1))
                    nc.scalar.activation(exps[mt][:], ps[:], AF.Exp, bias=shift_sb[:])

                # pass 1: mean + rowsum
                pvm = [ppv.tile([128, 512], F32, tag="pv", name="pvm")
                       for _ in range(CT)]
                pr = ppv.tile([128, 512], F32, tag="pv", name="pr")
                for mt in range(MT):
                    for ct in range(CT):
                        nc.tensor.matmul(
                            pvm[ct][:],
                            vt[mt][:, ct * 128:(ct + 1) * 128],
                            exps[mt][:],
                            start=(mt == 0), stop=(mt == MT - 1))
                    nc.tensor.matmul(pr[:], ones_r[:], exps[mt][:],
                                     start=(mt == 0), stop=(mt == MT - 1))
                for ct in range(CT):
                    asl = slice((nch * CT + ct) * 512, (nch * CT + ct + 1) * 512)
                    if mb == 0:
                        nc.scalar.activation(accm_all[:, asl], pvm[ct][:], AF.Copy)
                    else:
                        nc.vector.tensor_add(accm_all[:, asl], pvm[ct][:],
                                             accm_all[:, asl])
                if mb == 0:
                    nc.scalar.activation(accr[:, nsl], pr[:], AF.Copy)
                else:
                    nc.vector.tensor_add(accr[:, nsl], pr[:], accr[:, nsl])

                # pass 2: second moment
                pvs = [ppv.tile([128, 512], F32, tag="pv", name="pvs")
                       for _ in range(CT)]
                for mt in range(MT):
                    for ct in range(CT):
                        nc.tensor.matmul(
                            pvs[ct][:],
                            vtsq[mt][:, ct * 128:(ct + 1) * 128],
                            exps[mt][:],
                            start=(mt == 0), stop=(mt == MT - 1))
                for ct in range(CT):
                    asl = slice((nch * CT + ct) * 512, (nch * CT + ct + 1) * 512)
                    if mb == 0:
                        nc.scalar.activation(accs_all[:, asl], pvs[ct][:], AF.Copy)
                    else:
                        nc.vector.tensor_add(accs_all[:, asl], pvs[ct][:],
                                             accs_all[:, asl])
                if mb == NMB - 1:
                    emit_rpipe(nch)
            if mb == 1:
                emit_stats([0, 1])
            elif mb == 2:
                emit_stats([2, 3])

        for nch in range(NCH):
            emit_final(nch)

    nc.finalize()
    return nc


_NC = None
_last_in_maps = None


def _get_nc():
    global _NC
    if _NC is None:
        _NC = build_nc()
    return _NC


def kernel(content, style, content_key, style_key, Wf, bf, Wg, bg, Wh, bh):
    content = np.ascontiguousarray(np.asarray(content, np.float32))
    style = np.ascontiguousarray(np.asarray(style, np.float32))
    content_key = np.ascontiguousarray(np.asarray(content_key, np.float32))
    style_key = np.ascontiguousarray(np.asarray(style_key, np.float32))
    wfT = np.ascontiguousarray(np.asarray(Wf, np.float32).T)
    wgT = np.ascontiguousarray(np.asarray(Wg, np.float32).T)
    whT = np.ascontiguousarray(np.asarray(Wh, np.float32).T)
    bf = np.ascontiguousarray(np.asarray(bf, np.float32))
    bg = np.ascontiguousarray(np.asarray(bg, np.float32))
    bh = np.ascontiguousarray(np.asarray(bh, np.float32))

    nc = _get_nc()
    global _last_in_maps
    in_maps = []
    for core in range(8):
        b = core // 2
        h = core % 2
        nsl = slice(h * NL, (h + 1) * NL)
        in_maps.append({
            "ck": np.ascontiguousarray(content_key[b].reshape(C, N)[:, nsl]),
            "sk": np.ascontiguousarray(style_key[b].reshape(C, M)),
            "sv": np.ascontiguousarray(style[b].reshape(C, M)),
            "cont": np.ascontiguousarray(content[b].reshape(C, N)),
            "cont_nl": np.ascontiguousarray(content[b].reshape(C, N)[:, nsl]),
            "wfT": wfT, "wgT": wgT, "whT": whT,
            "bfv": bf, "bgv": bg, "bhv": bh,
        })
    _last_in_maps = in_maps
    res = run_bass_kernel_spmd(nc, in_maps, list(range(8)))

    out_full = np.empty((B, C, HH, WW), np.float32)
    for core in range(8):
        b = core // 2
        h = core % 2
        out_full[b].reshape(C, N)[:, h * NL:(h + 1) * NL] = res.results[core]["out"]
    return out_full
